# revision 22
# baseline (speedup 1.0000x reference)
"""WaveNet-style causal dilated conv stack (nn_CausalTemporalConv1d) on 8 TRN2 cores.

Strategy: data-parallel over batch (B=8 -> 1 batch element per core), params
replicated. Per core, everything stays on-chip except a DRAM-streamed cond
projection. Matmuls run in float32r (TF32-like, ~1e-4 relerr, bf16 speed);
the residual stream / GLU are f32r/f32 on DVE+ACT.

Per-core pipeline:
  x0 = start_w @ inp + start_b                     (fp32 matmul, K=80)
  c  = cond_w @ cond                               (fp32 matmul; biases folded
                                                    into per-layer bias vecs)
  c -> DRAM scratch, streamed back per layer       (saves 64KB/partition SBUF)
  for l in 0..7, d=2^l:
    A = dilated_causal_conv(x, w_in[l])            (f32r matmuls into PSUM,
                                                    boundary via partial-column
                                                    accumulation)
    pregate = A_gate + c_gate                      (DVE)
    sig     = Sigmoid(pregate + b_gate)            (ACT, per-partition bias)
    t_lin   = (A_lin + b_lin) + c_lin              (DVE scalar_tensor_tensor)
    acts    = t_lin * sig                          (DVE, f32r out)
    R = rs_w[l] @ acts                             (f32r matmuls)
    x' = (R_lin + rs_b_lin) + x                    (DVE stt, f32r out, ping-pong)
    skip += R_skip                                 (DVE; rs skip-biases folded
                                                    into end bias on host)
  out = end_w @ skip + end_b_eff                   (f32r matmul + ACT bias)
"""
import numpy as np

import concourse.bacc as bacc
import concourse.mybir as mybir
from concourse.tile import TileContext
from concourse.bass_utils import run_bass_kernel_spmd

F32 = mybir.dt.float32
F32R = mybir.dt.float32r
AF = mybir.ActivationFunctionType
ALU = mybir.AluOpType

N_CORES = 8
L = 8
K = 3
C = 256


def build_module(T=4096, iters=1):
    """Build the SPMD per-core Bass module. T must be a multiple of 512."""
    TW = 512
    NT = T // TW
    PAD = 256  # left zero-pad on x buffers: covers max tap shift (K-1)*d = 256
    nc = bacc.Bacc()

    inp_d = nc.declare_dram_parameter("inp", [80, T], F32, isOutput=False)
    cond_d = nc.declare_dram_parameter("cond", [128, 2, T], F32, isOutput=False)
    w_start_d = nc.declare_dram_parameter("w_start", [80, 256], F32, isOutput=False)
    w_cond_d = nc.declare_dram_parameter("w_cond", [128, 2, 512], F32, isOutput=False)
    w_in_d = nc.declare_dram_parameter("w_in", [L, 128, K, 2, 512], F32, isOutput=False)
    w_rs_d = nc.declare_dram_parameter("w_rs", [L - 1, 128, 2, 512], F32, isOutput=False)
    w_rsl_d = nc.declare_dram_parameter("w_rsl", [128, 2, 256], F32, isOutput=False)
    w_end_d = nc.declare_dram_parameter("w_end", [128, 2, 80], F32, isOutput=False)
    biases_d = nc.declare_dram_parameter("biases", [128, 51], F32, isOutput=False)
    ident_d = nc.declare_dram_parameter("ident", [128, 128], F32, isOutput=False)
    out_d = nc.declare_dram_parameter("out", [80, T], F32, isOutput=True)

    c_dram = nc.dram_tensor("c_scratch", [NT, 128, 4, TW], F32)

    def bias_col(sb, col, rows=128):
        return sb[0:rows, col:col + 1]

    with TileContext(nc) as tc:
        with tc.tile_pool(name="sb", bufs=1) as pool, \
             tc.tile_pool(name="ps", bufs=1, space="PSUM") as psum:

            # ---- Phase A: small weights + biases (outside timing loop) ----
            biases_sb = pool.tile([128, 51], F32, tag="biases")
            nc.sync.dma_start(out=biases_sb[:], in_=biases_d[:])

            w_start_sb = pool.tile([80, 256], F32, tag="w_start")
            nc.sync.dma_start(out=w_start_sb[:], in_=w_start_d[:])

            w_rsl_st = pool.tile([128, 2, 256], F32, tag="w_rsl_st")
            nc.sync.dma_start(out=w_rsl_st[:], in_=w_rsl_d[:])
            w_rsl_r = pool.tile([128, 2, 256], F32R, tag="w_rsl")
            nc.vector.tensor_copy(w_rsl_r[:], w_rsl_st[:])

            w_end_st = pool.tile([128, 2, 80], F32, tag="w_end_st")
            nc.sync.dma_start(out=w_end_st[:], in_=w_end_d[:])
            w_end_r = pool.tile([128, 2, 80], F32R, tag="w_end")
            nc.vector.tensor_copy(w_end_r[:], w_end_st[:])

            ident_st = pool.tile([128, 128], F32, tag="ident_st")
            nc.sync.dma_start(out=ident_st[:], in_=ident_d[:])
            ident_r = pool.tile([128, 128], F32R, tag="ident")
            nc.vector.tensor_copy(ident_r[:], ident_st[:])

            def body():
                # ---- Phase B: start conv (fp32), x0 -> xa ----
                inp_sb = pool.tile([80, T], F32, tag="skip", name="inp_sb")
                nc.sync.dma_start(out=inp_sb[:], in_=inp_d[:])
                zpad = pool.tile([128, 2, PAD], F32, tag="zpad")
                nc.vector.memset(zpad[:], 0.0)
                xa = pool.tile([128, 2, PAD + T], F32R, tag="xa")
                nc.vector.tensor_copy(xa[:, :, 0:PAD], zpad[:])

                for t in range(NT):
                    t0 = t * TW
                    p0 = psum.tile([128, 2, TW], F32, tag="Ag", name="p_start")
                    for cb in range(2):
                        nc.tensor.matmul(p0[:, cb, :],
                                         w_start_sb[:, cb * 128:(cb + 1) * 128],
                                         inp_sb[:, t0:t0 + TW],
                                         start=True, stop=True)
                        nc.vector.tensor_scalar_add(
                            xa[:, cb, PAD + t0:PAD + t0 + TW], p0[:, cb, :],
                            bias_col(biases_sb, 48 + cb))

                # ---- Phase C: cond conv (f32r) -> c_dram ----
                cond_sb = pool.tile([128, 2, T], F32, tag="xb", name="cond_sb")
                nc.sync.dma_start(out=cond_sb[:], in_=cond_d[:])
                w_cond_st = pool.tile([128, 2, 512], F32, tag="wst", name="w_cond_st")
                nc.sync.dma_start(out=w_cond_st[:], in_=w_cond_d[:])
                w_cond_r = pool.tile([128, 2, 512], F32R, tag="w_rs", bufs=2,
                                     name="w_cond_r")
                nc.vector.tensor_copy(w_cond_r[:], w_cond_st[:])

                for t in range(NT):
                    t0 = t * TW
                    cond_r = pool.tile([128, 2, TW], F32R, tag="c_gr", bufs=2,
                                       name="cond_r")
                    nc.vector.tensor_copy(cond_r[:], cond_sb[:, :, t0:t0 + TW])
                    for half in range(2):  # 0: lin co-blocks 0-1, 1: gate 2-3
                        pc = psum.tile([128, 2, TW], F32,
                                       tag=("Al" if half == 0 else "Ag"),
                                       name="p_cond")
                        for cb2 in range(2):
                            co = half * 2 + cb2
                            for ch in range(2):
                                nc.tensor.matmul(
                                    pc[:, cb2, :],
                                    w_cond_r[:, ch, co * 128:(co + 1) * 128],
                                    cond_r[:, ch, :],
                                    start=(ch == 0), stop=(ch == 1))
                        if half == 0:
                            c_out_l = pool.tile([128, 2, TW], F32, tag="c_lin",
                                                bufs=2, name="c_out_l")
                            nc.scalar.activation(c_out_l[:], pc[:], AF.Copy)
                            nc.sync.dma_start(out=c_dram[t, :, 0:2, :],
                                              in_=c_out_l[:])
                        else:
                            c_out_g = pool.tile([128, 2, TW], F32R, tag="c_gst",
                                                bufs=2, name="c_out_g")
                            nc.scalar.activation(c_out_g[:], pc[:], AF.Copy)
                            nc.sync.dma_start(
                                out=c_dram[t, :, 2:4, :].bitcast(F32R),
                                in_=c_out_g[:])

                # ---- Phase D: layers ----
                xb = pool.tile([128, 2, PAD + T], F32R, tag="xb", name="xb")
                nc.vector.tensor_copy(xb[:, :, 0:PAD], zpad[:])
                skip = pool.tile([128, 2, T], F32R, tag="skip", name="skip")

                # rs-conv of tile t is emitted one tile later (after the next
                # tile's in-conv matmuls) so PE fills the GLU latency.
                pending_rs = [None]

                def flush_rs():
                    if pending_rs[0] is not None:
                        pending_rs[0]()
                        pending_rs[0] = None

                for l in range(L):
                    d = 2 ** l
                    w_st = pool.tile([128, K, 2, 512], F32, tag="wst", name="w_in_st")
                    nc.sync.dma_start(out=w_st[:], in_=w_in_d[l])
                    w_in_r = pool.tile([128, K, 2, 512], F32R, tag="w_in", bufs=2,
                                       name="w_in_r")
                    nc.vector.tensor_copy(w_in_r[:], w_st[:])
                    if l < L - 1:
                        w_rs_st = pool.tile([128, 2, 512], F32, tag="wst",
                                            name="w_rs_st")
                        nc.sync.dma_start(out=w_rs_st[:], in_=w_rs_d[l])
                        w_rs_r = pool.tile([128, 2, 512], F32R, tag="w_rs", bufs=2,
                                           name="w_rs_r")
                        nc.vector.tensor_copy(w_rs_r[:], w_rs_st[:])

                    xp, xn = (xa, xb) if l % 2 == 0 else (xb, xa)

                    for t in range(NT):
                        t0 = t * TW
                        ctile_l = pool.tile([128, 2, TW], F32, tag="c_lin",
                                            name="c_in_l", bufs=2)
                        nc.sync.dma_start(out=ctile_l[:], in_=c_dram[t, :, 0:2, :])
                        c_g_st = pool.tile([128, 2, TW], F32, tag="c_gst",
                                           name="c_g_st", bufs=2)
                        nc.sync.dma_start(out=c_g_st[:], in_=c_dram[t, :, 2:4, :])
                        c_g_r = pool.tile([128, 2, TW], F32R, tag="c_gr",
                                          name="c_g_r", bufs=2)
                        nc.scalar.activation(c_g_r[:], c_g_st[:], AF.Copy)

                        def conv_in(ps, co, extra_c=None):
                            """dilated conv for out-channel block co into ps;
                            extra_c: f32r [128, TW] tile added via identity mm."""
                            # tap k reads x[:, t0 + (k-2)*d : +TW] (zero-padded)
                            last = 5 if extra_c is None else 6
                            for i, (k, ch) in enumerate(
                                    [(k, ch) for k in (2, 1, 0) for ch in range(2)]):
                                shift = (2 - k) * d
                                nc.tensor.matmul(
                                    ps,
                                    w_in_r[:, k, ch, co * 128:(co + 1) * 128],
                                    xp[:, ch, PAD + t0 - shift: PAD + t0 - shift + TW],
                                    start=(i == 0), stop=(i == last))
                            if extra_c is not None:
                                nc.tensor.matmul(ps, ident_r[:], extra_c,
                                                 start=False, stop=True)

                        Ag = psum.tile([128, 2, TW], F32, tag="Ag", name="Ag")
                        for cb2 in range(2):
                            conv_in(Ag[:, cb2, :], 2 + cb2, extra_c=c_g_r[:, cb2, :])
                        sig = pool.tile([128, 2, TW], F32, tag="sig", bufs=2,
                                        name="sig")
                        for cb in range(2):
                            nc.scalar.activation(sig[:, cb, :], Ag[:, cb, :],
                                                 AF.Sigmoid,
                                                 bias=bias_col(biases_sb, l * 6 + 2 + cb))

                        Al = psum.tile([128, 2, TW], F32, tag="Al", name="Al")
                        for cb2 in range(2):
                            conv_in(Al[:, cb2, :], cb2)
                        t_lin = pool.tile([128, 2, TW], F32, tag="t_lin", bufs=2,
                                          name="t_lin")
                        for cb in range(2):
                            nc.vector.scalar_tensor_tensor(
                                t_lin[:, cb, :], Al[:, cb, :],
                                bias_col(biases_sb, l * 6 + cb),
                                ctile_l[:, cb, :], op0=ALU.add, op1=ALU.add)
                        acts = pool.tile([128, 2, TW], F32R, tag="acts", bufs=2,
                                         name="acts")
                        nc.vector.tensor_tensor(acts[:], t_lin[:], sig[:],
                                                op=ALU.mult)

                        # ---- rs conv (deferred one tile) ----
                        def make_rs(l=l, t0=t0, acts=acts, xp=xp, xn=xn,
                                    w_rs=(w_rs_r if l < L - 1 else None)):
                            def emit():
                                if w_rs is not None:
                                    Rxy = psum.tile([128, 2, TW], F32, tag="Rxy",
                                                    name="Rxy")
                                    Rsk = psum.tile([128, 2, TW], F32, tag="Rsk",
                                                    name="Rsk")
                                    for co in range(4):
                                        ps = (Rxy if co < 2 else Rsk)[:, co % 2, :]
                                        for ch in range(2):
                                            nc.tensor.matmul(
                                                ps,
                                                w_rs[:, ch, co * 128:(co + 1) * 128],
                                                acts[:, ch, :],
                                                start=(ch == 0), stop=(ch == 1))
                                    for cb in range(2):
                                        nc.vector.scalar_tensor_tensor(
                                            xn[:, cb, PAD + t0:PAD + t0 + TW],
                                            Rxy[:, cb, :],
                                            bias_col(biases_sb, l * 6 + 4 + cb),
                                            xp[:, cb, PAD + t0:PAD + t0 + TW],
                                            op0=ALU.add, op1=ALU.add)
                                    if l == 0:
                                        nc.vector.tensor_copy(
                                            skip[:, :, t0:t0 + TW], Rsk[:])
                                    else:
                                        nc.vector.tensor_tensor(
                                            skip[:, :, t0:t0 + TW],
                                            skip[:, :, t0:t0 + TW],
                                            Rsk[:], op=ALU.add)
                                else:
                                    Rsk = psum.tile([128, 2, TW], F32, tag="Rsk",
                                                    name="Rsk")
                                    for co in range(2):
                                        for ch in range(2):
                                            nc.tensor.matmul(
                                                Rsk[:, co, :],
                                                w_rsl_r[:, ch, co * 128:(co + 1) * 128],
                                                acts[:, ch, :],
                                                start=(ch == 0), stop=(ch == 1))
                                    nc.vector.tensor_tensor(
                                        skip[:, :, t0:t0 + TW],
                                        skip[:, :, t0:t0 + TW],
                                        Rsk[:], op=ALU.add)
                            return emit

                        cur_rs = make_rs()
                        flush_rs()
                        pending_rs[0] = cur_rs

                flush_rs()

                # ---- Phase E: end conv ----
                for t in range(NT):
                    t0 = t * TW
                    pe = psum.tile([128, 2, TW], F32, tag="Ag", name="p_end")
                    for ch in range(2):
                        nc.tensor.matmul(pe[0:80, 0, :], w_end_r[:, ch, :],
                                         skip[:, ch, t0:t0 + TW],
                                         start=(ch == 0), stop=(ch == 1))
                    o_sb = pool.tile([80, TW], F32, tag="ostage", bufs=2,
                                     name="o_sb")
                    nc.scalar.activation(o_sb[:], pe[0:80, 0, :], AF.Identity,
                                         bias=bias_col(biases_sb, 50, rows=80))
                    nc.sync.dma_start(out=out_d[:, t0:t0 + TW], in_=o_sb[:])

            if iters == 1:
                body()
            else:
                with tc.For_i(0, iters):
                    body()

    nc.finalize()
    return nc


def prep_inputs(inp, cond, start_w, start_b, cond_w, cond_b, in_w, in_b,
                rs_w, rs_b, rs_w_last, rs_b_last, end_w, end_b):
    """Host-side weight/bias re-layout. Returns per-core in_maps."""
    f32 = np.float32
    T = inp.shape[-1]
    b_total = (in_b + cond_b[None, :]).astype(f32)          # [L, 512]
    b_lin, b_gate = b_total[:, :C], b_total[:, C:]
    rs_lin_b = rs_b[:, :C].astype(f32)                      # [L-1, 256]
    skip_bias = rs_b[:, C:].sum(0) + rs_b_last              # [256]
    end_b_eff = (end_b + end_w[:, :, 0] @ skip_bias).astype(f32)

    biases = np.zeros((128, 51), f32)
    for l in range(L):
        for ch in range(2):
            biases[:, l * 6 + ch] = b_lin[l, ch * 128:(ch + 1) * 128]
            biases[:, l * 6 + 2 + ch] = b_gate[l, ch * 128:(ch + 1) * 128]
            if l < L - 1:
                biases[:, l * 6 + 4 + ch] = rs_lin_b[l, ch * 128:(ch + 1) * 128]
    biases[:, 48] = start_b[:128]
    biases[:, 49] = start_b[128:]
    biases[:80, 50] = end_b_eff

    w_start = np.ascontiguousarray(start_w[:, :, 0].T, dtype=f32)       # [80,256]
    w_cond = np.ascontiguousarray(
        cond_w[:, :, 0].T.reshape(2, 128, 512).transpose(1, 0, 2), dtype=f32)
    w_in = np.ascontiguousarray(
        in_w.transpose(0, 2, 3, 1).reshape(L, 2, 128, K, 512)
            .transpose(0, 2, 3, 1, 4), dtype=f32)                        # [L,128,K,2,512]
    w_rs = np.ascontiguousarray(
        rs_w[:, :, :, 0].transpose(0, 2, 1).reshape(L - 1, 2, 128, 512)
            .transpose(0, 2, 1, 3), dtype=f32)                           # [L-1,128,2,512]
    w_rsl = np.ascontiguousarray(
        rs_w_last[:, :, 0].T.reshape(2, 128, 256).transpose(1, 0, 2), dtype=f32)
    w_end = np.ascontiguousarray(
        end_w[:, :, 0].T.reshape(2, 128, 80).transpose(1, 0, 2), dtype=f32)

    shared = dict(w_start=w_start, w_cond=w_cond, w_in=w_in, w_rs=w_rs,
                  w_rsl=w_rsl, w_end=w_end, biases=biases,
                  ident=np.eye(128, dtype=f32))
    in_maps = []
    for b in range(inp.shape[0]):
        m = dict(shared)
        m["inp"] = np.ascontiguousarray(inp[b], dtype=f32)
        m["cond"] = np.ascontiguousarray(
            cond[b].reshape(2, 128, T).transpose(1, 0, 2), dtype=f32)
        in_maps.append(m)
    return in_maps


_NC_CACHE = {}


def kernel(**inputs) -> np.ndarray:
    T = inputs["inp"].shape[-1]
    in_maps = prep_inputs(**inputs)
    key = (T, 1)
    if key not in _NC_CACHE:
        _NC_CACHE[key] = build_module(T=T, iters=1)
    nc = _NC_CACHE[key]
    res = run_bass_kernel_spmd(nc, in_maps, list(range(N_CORES))).results
    return np.stack([r["out"] for r in res]).astype(np.float32)


# revision 28
# speedup vs baseline: 1.0921x; 1.0921x over previous
"""WaveNet-style causal dilated conv stack (nn_CausalTemporalConv1d) on 8 TRN2 cores.

Strategy: data-parallel over batch (B=8 -> 1 batch element per core), params
replicated. Per core, everything stays on-chip except a DRAM-streamed cond
projection. Matmuls run in float32r (TF32-like, ~1e-4 relerr, bf16 speed);
the residual stream / GLU are f32r/f32 on DVE+ACT.

Per-core pipeline:
  x0 = start_w @ inp + start_b                     (fp32 matmul, K=80)
  c  = cond_w @ cond                               (fp32 matmul; biases folded
                                                    into per-layer bias vecs)
  c -> DRAM scratch, streamed back per layer       (saves 64KB/partition SBUF)
  for l in 0..7, d=2^l:
    A = dilated_causal_conv(x, w_in[l])            (f32r matmuls into PSUM,
                                                    boundary via partial-column
                                                    accumulation)
    pregate = A_gate + c_gate                      (DVE)
    sig     = Sigmoid(pregate + b_gate)            (ACT, per-partition bias)
    t_lin   = (A_lin + b_lin) + c_lin              (DVE scalar_tensor_tensor)
    acts    = t_lin * sig                          (DVE, f32r out)
    R = rs_w[l] @ acts                             (f32r matmuls)
    x' = (R_lin + rs_b_lin) + x                    (DVE stt, f32r out, ping-pong)
    skip += R_skip                                 (DVE; rs skip-biases folded
                                                    into end bias on host)
  out = end_w @ skip + end_b_eff                   (f32r matmul + ACT bias)
"""
import numpy as np

import concourse.bass as bass
import concourse.bacc as bacc
import concourse.mybir as mybir
from concourse.tile import TileContext
from concourse.bass_utils import run_bass_kernel_spmd

F32 = mybir.dt.float32
F32R = mybir.dt.float32r
AF = mybir.ActivationFunctionType
ALU = mybir.AluOpType

N_CORES = 8
L = 8
K = 3
C = 256


def build_module(T=4096, iters=1, use_identity=False, defer_rs=False):
    """Build the SPMD per-core Bass module. T must be a multiple of 512."""
    TW = 512
    NT = T // TW
    PAD = 256  # left zero-pad on x buffers: covers max tap shift (K-1)*d = 256
    nc = bacc.Bacc()

    inp_d = nc.declare_dram_parameter("inp", [80, T], F32, isOutput=False)
    cond_d = nc.declare_dram_parameter("cond", [128, 2, T], F32, isOutput=False)
    w_start_d = nc.declare_dram_parameter("w_start", [80, 256], F32, isOutput=False)
    w_cond_d = nc.declare_dram_parameter("w_cond", [128, 2, 512], F32, isOutput=False)
    w_in_d = nc.declare_dram_parameter("w_in", [L, 128, K, 2, 512], F32, isOutput=False)
    w_rs_d = nc.declare_dram_parameter("w_rs", [L - 1, 128, 2, 512], F32, isOutput=False)
    w_rsl_d = nc.declare_dram_parameter("w_rsl", [128, 2, 256], F32, isOutput=False)
    w_end_d = nc.declare_dram_parameter("w_end", [128, 2, 80], F32, isOutput=False)
    biases_d = nc.declare_dram_parameter("biases", [128, 51], F32, isOutput=False)
    ident_d = nc.declare_dram_parameter("ident", [128, 128], F32, isOutput=False)
    if iters == "runtime":
        niter_d = nc.declare_dram_parameter("niter", [1, 1], mybir.dt.int32,
                                            isOutput=False)
    out_d = nc.declare_dram_parameter("out", [80, T], F32, isOutput=True)

    c_dram = nc.dram_tensor("c_scratch", [NT, 128, 4, TW], F32)

    def bias_col(sb, col, rows=128):
        return sb[0:rows, col:col + 1]

    with TileContext(nc) as tc:
        with tc.tile_pool(name="sb", bufs=1) as pool, \
             tc.tile_pool(name="ps", bufs=1, space="PSUM") as psum:

            # ---- Phase A: small weights + biases (outside timing loop) ----
            biases_sb = pool.tile([128, 51], F32, tag="biases")
            nc.sync.dma_start(out=biases_sb[:], in_=biases_d[:])

            w_start_sb = pool.tile([80, 256], F32, tag="w_start")
            nc.sync.dma_start(out=w_start_sb[:], in_=w_start_d[:])

            w_rsl_st = pool.tile([128, 2, 256], F32, tag="w_rsl_st")
            nc.sync.dma_start(out=w_rsl_st[:], in_=w_rsl_d[:])
            w_rsl_r = pool.tile([128, 2, 256], F32R, tag="w_rsl")
            nc.vector.tensor_copy(w_rsl_r[:], w_rsl_st[:])

            w_end_st = pool.tile([128, 2, 80], F32, tag="w_end_st")
            nc.sync.dma_start(out=w_end_st[:], in_=w_end_d[:])
            w_end_r = pool.tile([128, 2, 80], F32R, tag="w_end")
            nc.vector.tensor_copy(w_end_r[:], w_end_st[:])

            ident_st = pool.tile([128, 128], F32, tag="ident_st")
            nc.sync.dma_start(out=ident_st[:], in_=ident_d[:])
            ident_r = pool.tile([128, 128], F32R, tag="ident")
            nc.vector.tensor_copy(ident_r[:], ident_st[:])

            def body():
                # ---- Phase B: start conv (fp32), x0 -> xa ----
                inp_sb = pool.tile([80, T], F32, tag="skip", name="inp_sb")
                nc.sync.dma_start(out=inp_sb[:], in_=inp_d[:])
                zpad = pool.tile([128, 2, PAD], F32, tag="zpad")
                nc.vector.memset(zpad[:], 0.0)
                xa = pool.tile([128, 2, PAD + T], F32R, tag="xa")
                nc.vector.tensor_copy(xa[:, :, 0:PAD], zpad[:])

                for t in range(NT):
                    t0 = t * TW
                    p0 = psum.tile([128, 2, TW], F32, tag="Ag", name="p_start")
                    for cb in range(2):
                        nc.tensor.matmul(p0[:, cb, :],
                                         w_start_sb[:, cb * 128:(cb + 1) * 128],
                                         inp_sb[:, t0:t0 + TW],
                                         start=True, stop=True)
                        nc.vector.tensor_scalar_add(
                            xa[:, cb, PAD + t0:PAD + t0 + TW], p0[:, cb, :],
                            bias_col(biases_sb, 48 + cb))

                # ---- Phase C: cond conv (f32r) -> c_dram ----
                cond_sb = pool.tile([128, 2, T], F32, tag="xb", name="cond_sb")
                nc.sync.dma_start(out=cond_sb[:], in_=cond_d[:])
                w_cond_st = pool.tile([128, 2, 512], F32, tag="wst", name="w_cond_st")
                nc.sync.dma_start(out=w_cond_st[:], in_=w_cond_d[:])
                w_cond_r = pool.tile([128, 2, 512], F32R, tag="w_rs", bufs=2,
                                     name="w_cond_r")
                nc.vector.tensor_copy(w_cond_r[:], w_cond_st[:])

                for t in range(NT):
                    t0 = t * TW
                    cond_r = pool.tile([128, 2, TW], F32R, tag="c_gr", bufs=2,
                                       name="cond_r")
                    nc.vector.tensor_copy(cond_r[:], cond_sb[:, :, t0:t0 + TW])
                    for half in range(2):  # 0: lin co-blocks 0-1, 1: gate 2-3
                        pc = psum.tile([128, 2, TW], F32,
                                       tag=("Al" if half == 0 else "Ag"),
                                       name="p_cond")
                        for cb2 in range(2):
                            co = half * 2 + cb2
                            for ch in range(2):
                                nc.tensor.matmul(
                                    pc[:, cb2, :],
                                    w_cond_r[:, ch, co * 128:(co + 1) * 128],
                                    cond_r[:, ch, :],
                                    start=(ch == 0), stop=(ch == 1))
                        if half == 0:
                            c_out_l = pool.tile([128, 2, TW], F32, tag="c_lin",
                                                bufs=2, name="c_out_l")
                            nc.scalar.activation(c_out_l[:], pc[:], AF.Copy)
                            nc.sync.dma_start(out=c_dram[t, :, 0:2, :],
                                              in_=c_out_l[:])
                        else:
                            c_out_g = pool.tile([128, 2, TW], F32R, tag="c_gst",
                                                bufs=2, name="c_out_g")
                            nc.scalar.activation(c_out_g[:], pc[:], AF.Copy)
                            nc.sync.dma_start(
                                out=c_dram[t, :, 2:4, :].bitcast(F32R),
                                in_=c_out_g[:])

                # ---- Phase D: layers ----
                xb = pool.tile([128, 2, PAD + T], F32R, tag="xb", name="xb")
                nc.vector.tensor_copy(xb[:, :, 0:PAD], zpad[:])
                skip = pool.tile([128, 2, T], F32R, tag="skip", name="skip")

                # rs-conv of tile t is emitted one tile later (after the next
                # tile's in-conv matmuls) so PE fills the GLU latency.
                pending_rs = [None]

                def flush_rs():
                    if pending_rs[0] is not None:
                        pending_rs[0]()
                        pending_rs[0] = None

                for l in range(L):
                    d = 2 ** l
                    w_st = pool.tile([128, K, 2, 512], F32, tag="wst", name="w_in_st")
                    nc.sync.dma_start(out=w_st[:], in_=w_in_d[l])
                    w_in_r = pool.tile([128, K, 2, 512], F32R, tag="w_in", bufs=2,
                                       name="w_in_r")
                    nc.vector.tensor_copy(w_in_r[:], w_st[:])
                    if l < L - 1:
                        w_rs_st = pool.tile([128, 2, 512], F32, tag="wst",
                                            name="w_rs_st")
                        nc.sync.dma_start(out=w_rs_st[:], in_=w_rs_d[l])
                        w_rs_r = pool.tile([128, 2, 512], F32R, tag="w_rs", bufs=2,
                                           name="w_rs_r")
                        nc.vector.tensor_copy(w_rs_r[:], w_rs_st[:])

                    xp, xn = (xa, xb) if l % 2 == 0 else (xb, xa)

                    for t in range(NT):
                        t0 = t * TW
                        ctile_l = pool.tile([128, 2, TW], F32, tag="c_lin",
                                            name="c_in_l", bufs=2)
                        nc.sync.dma_start(out=ctile_l[:], in_=c_dram[t, :, 0:2, :])
                        c_g_st = pool.tile([128, 2, TW], F32, tag="c_gst",
                                           name="c_g_st", bufs=2)
                        nc.sync.dma_start(out=c_g_st[:], in_=c_dram[t, :, 2:4, :])
                        c_g_r = pool.tile([128, 2, TW], F32R, tag="c_gr",
                                          name="c_g_r", bufs=2)
                        nc.scalar.activation(c_g_r[:], c_g_st[:], AF.Copy)

                        def conv_in(ps, co, extra_c=None):
                            """dilated conv for out-channel block co into ps;
                            extra_c: f32r [128, TW] tile added via identity mm."""
                            # tap k reads x[:, t0 + (k-2)*d : +TW] (zero-padded)
                            last = 5 if extra_c is None else 6
                            for i, (k, ch) in enumerate(
                                    [(k, ch) for k in (2, 1, 0) for ch in range(2)]):
                                shift = (2 - k) * d
                                nc.tensor.matmul(
                                    ps,
                                    w_in_r[:, k, ch, co * 128:(co + 1) * 128],
                                    xp[:, ch, PAD + t0 - shift: PAD + t0 - shift + TW],
                                    start=(i == 0), stop=(i == last))
                            if extra_c is not None:
                                nc.tensor.matmul(ps, ident_r[:], extra_c,
                                                 start=False, stop=True)

                        Ag = psum.tile([128, 2, TW], F32, tag="Ag", name="Ag")
                        for cb2 in range(2):
                            conv_in(Ag[:, cb2, :], 2 + cb2,
                                    extra_c=(c_g_r[:, cb2, :] if use_identity
                                             else None))
                        sig = pool.tile([128, 2, TW], F32, tag="sig", bufs=2,
                                        name="sig")
                        if not use_identity:
                            pregate = pool.tile([128, 2, TW], F32, tag="t_lin",
                                                bufs=2, name="pregate")
                            nc.vector.tensor_tensor(pregate[:], Ag[:], c_g_r[:],
                                                    op=ALU.add)
                            sig_src = pregate
                        else:
                            sig_src = Ag
                        for cb in range(2):
                            nc.scalar.activation(sig[:, cb, :], sig_src[:, cb, :],
                                                 AF.Sigmoid,
                                                 bias=bias_col(biases_sb, l * 6 + 2 + cb))

                        Al = psum.tile([128, 2, TW], F32, tag="Al", name="Al")
                        for cb2 in range(2):
                            conv_in(Al[:, cb2, :], cb2)
                        t_lin = pool.tile([128, 2, TW], F32, tag="t_lin", bufs=2,
                                          name="t_lin")
                        for cb in range(2):
                            nc.vector.scalar_tensor_tensor(
                                t_lin[:, cb, :], Al[:, cb, :],
                                bias_col(biases_sb, l * 6 + cb),
                                ctile_l[:, cb, :], op0=ALU.add, op1=ALU.add)
                        acts = pool.tile([128, 2, TW], F32R, tag="acts", bufs=2,
                                         name="acts")
                        nc.vector.tensor_tensor(acts[:], t_lin[:], sig[:],
                                                op=ALU.mult)

                        # ---- rs conv (deferred one tile) ----
                        def make_rs(l=l, t0=t0, acts=acts, xp=xp, xn=xn,
                                    w_rs=(w_rs_r if l < L - 1 else None)):
                            def emit():
                                if w_rs is not None:
                                    Rxy = psum.tile([128, 2, TW], F32, tag="Rxy",
                                                    name="Rxy")
                                    Rsk = psum.tile([128, 2, TW], F32, tag="Rsk",
                                                    name="Rsk")
                                    for co in range(4):
                                        ps = (Rxy if co < 2 else Rsk)[:, co % 2, :]
                                        for ch in range(2):
                                            nc.tensor.matmul(
                                                ps,
                                                w_rs[:, ch, co * 128:(co + 1) * 128],
                                                acts[:, ch, :],
                                                start=(ch == 0), stop=(ch == 1))
                                    for cb in range(2):
                                        nc.vector.scalar_tensor_tensor(
                                            xn[:, cb, PAD + t0:PAD + t0 + TW],
                                            Rxy[:, cb, :],
                                            bias_col(biases_sb, l * 6 + 4 + cb),
                                            xp[:, cb, PAD + t0:PAD + t0 + TW],
                                            op0=ALU.add, op1=ALU.add)
                                    if l == 0:
                                        nc.vector.tensor_copy(
                                            skip[:, :, t0:t0 + TW], Rsk[:])
                                    else:
                                        nc.vector.tensor_tensor(
                                            skip[:, :, t0:t0 + TW],
                                            skip[:, :, t0:t0 + TW],
                                            Rsk[:], op=ALU.add)
                                else:
                                    Rsk = psum.tile([128, 2, TW], F32, tag="Rsk",
                                                    name="Rsk")
                                    for co in range(2):
                                        for ch in range(2):
                                            nc.tensor.matmul(
                                                Rsk[:, co, :],
                                                w_rsl_r[:, ch, co * 128:(co + 1) * 128],
                                                acts[:, ch, :],
                                                start=(ch == 0), stop=(ch == 1))
                                    nc.vector.tensor_tensor(
                                        skip[:, :, t0:t0 + TW],
                                        skip[:, :, t0:t0 + TW],
                                        Rsk[:], op=ALU.add)
                            return emit

                        cur_rs = make_rs()
                        if defer_rs:
                            flush_rs()
                            pending_rs[0] = cur_rs
                        else:
                            cur_rs()

                flush_rs()

                # ---- Phase E: end conv ----
                for t in range(NT):
                    t0 = t * TW
                    pe = psum.tile([128, 2, TW], F32, tag="Ag", name="p_end")
                    for ch in range(2):
                        nc.tensor.matmul(pe[0:80, 0, :], w_end_r[:, ch, :],
                                         skip[:, ch, t0:t0 + TW],
                                         start=(ch == 0), stop=(ch == 1))
                    o_sb = pool.tile([80, TW], F32, tag="ostage", bufs=2,
                                     name="o_sb")
                    nc.scalar.activation(o_sb[:], pe[0:80, 0, :], AF.Identity,
                                         bias=bias_col(biases_sb, 50, rows=80))
                    nc.sync.dma_start(out=out_d[:, t0:t0 + TW], in_=o_sb[:])

            if iters == 1:
                body()
            elif iters == "runtime":
                niter_sb = pool.tile([1, 1], mybir.dt.int32, tag="niter_sb")
                nc.sync.dma_start(out=niter_sb[:], in_=niter_d[:])
                regs = []
                for et in mybir.ALL_ENGINES:
                    reg = nc.alloc_register(et, f"niter_{et.name}")
                    nc.engines[et].reg_load(reg, niter_sb[0:1, 0:1])
                    regs.append(reg)
                end_sv = nc.snap(bass.RegisterHandles(regs), donate=True,
                                 min_val=1, max_val=1 << 20)
                with tc.For_i(0, end_sv):
                    body()
            else:
                with tc.For_i(0, iters):
                    body()

    nc.finalize()
    return nc


def prep_inputs(inp, cond, start_w, start_b, cond_w, cond_b, in_w, in_b,
                rs_w, rs_b, rs_w_last, rs_b_last, end_w, end_b):
    """Host-side weight/bias re-layout. Returns per-core in_maps."""
    f32 = np.float32
    T = inp.shape[-1]
    b_total = (in_b + cond_b[None, :]).astype(f32)          # [L, 512]
    b_lin, b_gate = b_total[:, :C], b_total[:, C:]
    rs_lin_b = rs_b[:, :C].astype(f32)                      # [L-1, 256]
    skip_bias = rs_b[:, C:].sum(0) + rs_b_last              # [256]
    end_b_eff = (end_b + end_w[:, :, 0] @ skip_bias).astype(f32)

    biases = np.zeros((128, 51), f32)
    for l in range(L):
        for ch in range(2):
            biases[:, l * 6 + ch] = b_lin[l, ch * 128:(ch + 1) * 128]
            biases[:, l * 6 + 2 + ch] = b_gate[l, ch * 128:(ch + 1) * 128]
            if l < L - 1:
                biases[:, l * 6 + 4 + ch] = rs_lin_b[l, ch * 128:(ch + 1) * 128]
    biases[:, 48] = start_b[:128]
    biases[:, 49] = start_b[128:]
    biases[:80, 50] = end_b_eff

    w_start = np.ascontiguousarray(start_w[:, :, 0].T, dtype=f32)       # [80,256]
    w_cond = np.ascontiguousarray(
        cond_w[:, :, 0].T.reshape(2, 128, 512).transpose(1, 0, 2), dtype=f32)
    w_in = np.ascontiguousarray(
        in_w.transpose(0, 2, 3, 1).reshape(L, 2, 128, K, 512)
            .transpose(0, 2, 3, 1, 4), dtype=f32)                        # [L,128,K,2,512]
    w_rs = np.ascontiguousarray(
        rs_w[:, :, :, 0].transpose(0, 2, 1).reshape(L - 1, 2, 128, 512)
            .transpose(0, 2, 1, 3), dtype=f32)                           # [L-1,128,2,512]
    w_rsl = np.ascontiguousarray(
        rs_w_last[:, :, 0].T.reshape(2, 128, 256).transpose(1, 0, 2), dtype=f32)
    w_end = np.ascontiguousarray(
        end_w[:, :, 0].T.reshape(2, 128, 80).transpose(1, 0, 2), dtype=f32)

    shared = dict(w_start=w_start, w_cond=w_cond, w_in=w_in, w_rs=w_rs,
                  w_rsl=w_rsl, w_end=w_end, biases=biases,
                  ident=np.eye(128, dtype=f32))
    in_maps = []
    for b in range(inp.shape[0]):
        m = dict(shared)
        m["inp"] = np.ascontiguousarray(inp[b], dtype=f32)
        m["cond"] = np.ascontiguousarray(
            cond[b].reshape(2, 128, T).transpose(1, 0, 2), dtype=f32)
        in_maps.append(m)
    return in_maps


_NC_CACHE = {}


def _run(inputs) -> np.ndarray:
    T = inputs["inp"].shape[-1]
    in_maps = prep_inputs(**inputs)
    key = (T, 1)
    if key not in _NC_CACHE:
        _NC_CACHE[key] = build_module(T=T, iters=1)
    nc = _NC_CACHE[key]
    res = run_bass_kernel_spmd(nc, in_maps, list(range(N_CORES))).results
    return np.stack([r["out"] for r in res]).astype(np.float32)


def kernel(**inputs) -> np.ndarray:
    """Run on hardware; on a device fault, retry in fresh subprocesses
    (an NRT_EXEC_UNIT_UNRECOVERABLE poisons the calling process's backend)."""
    try:
        return _run(inputs)
    except Exception as e:  # noqa: BLE001 - device faults surface as many types
        import subprocess, sys, tempfile, os, traceback
        traceback.print_exc()
        print(f"kernel: in-process run failed ({type(e).__name__}); "
              f"retrying in subprocess", file=sys.stderr, flush=True)
        last = None
        for attempt in range(3):
            with tempfile.TemporaryDirectory() as td:
                inp_path = os.path.join(td, "in.npz")
                out_path = os.path.join(td, "out.npy")
                np.savez(inp_path, **inputs)
                r = subprocess.run(
                    [sys.executable, os.path.abspath(__file__),
                     "--worker", inp_path, out_path],
                    capture_output=True, text=True, timeout=3600)
                if r.returncode == 0 and os.path.exists(out_path):
                    return np.load(out_path)
                last = r.stderr[-2000:]
                print(f"kernel: subprocess attempt {attempt} failed:\n{last}",
                      file=sys.stderr, flush=True)
        raise RuntimeError(f"kernel: all retries failed; last stderr:\n{last}")


if __name__ == "__main__":
    import sys as _sys
    if len(_sys.argv) == 4 and _sys.argv[1] == "--worker":
        _data = np.load(_sys.argv[2])
        _out = _run({k: _data[k] for k in _data.files})
        np.save(_sys.argv[3], _out)
